# revision 1
# baseline (speedup 1.0000x reference)
"""Multi-head causal self-attention (B=4, S=2048, H=16, D=128) on 8 TRN2 cores.

Sharding: core c = (batch b = c//2, head-group g = c%2 of 8 heads).
Each core computes Q/K/V projections for its 8 heads, causal attention,
and the partial output projection (its heads' rows of Wo). The host sums
the two head-group partials per batch and adds bo (the unshard step).

Matmul dtype is fp32r (1 PE cycle/row vs fp32's 4): every SBUF operand
feeding a matmul is produced *as* fp32r (the BIR verifier requires the
producing instruction to round); DMA-loaded tensors bounce through a
one-time DVE copy. MODE="f32" falls back to plain fp32 matmuls.

Per (head, q-block of 512): S^T tiles [128 k, 512 q] = K_kt @ Q^T on PE,
exp on ACT (scale=1/sqrt(128); no max subtraction — |scores*scale| stays
O(1) for this input distribution), causal diag masks on DVE, then PV
accumulates ctx^T [128 d, 512 q] in PSUM with V tiles stationary; a
ones-column matmul accumulates the softmax denominator [1, 512].
Normalization: reciprocal + rank-1 broadcast matmul + one DVE multiply.
Output projection contracts heads with Wo_h stationary producing
out^T [128 e, 512 q]; DVE accumulates across heads into out_acc,
DMA'd back as out_t [128, 2048] (host transposes)."""

import os
import sys

import numpy as np

NUM_HEADS = 16
D = 128
B = 4
S = 2048
HPC = 8  # heads per core
N_CORES = 8
SCALE = 1.0 / np.sqrt(128.0)
MODE = os.environ.get("MHA_MODE", "f32r")  # "f32r" | "f32"

_CACHE = {}


def _import_concourse():
    if "/opt/trn_rl_repo" not in sys.path and os.path.isdir("/opt/trn_rl_repo"):
        sys.path.insert(0, "/opt/trn_rl_repo")


def _build_nc():
    _import_concourse()
    from contextlib import ExitStack

    import concourse.mybir as mybir
    import concourse.tile as tile
    from concourse import bacc

    F32 = mybir.dt.float32
    MM = mybir.dt.float32r if MODE == "f32r" else F32
    EXP = mybir.ActivationFunctionType.Exp

    nc = bacc.Bacc(trn_type="TRN2", target_bir_lowering=False, debug=False)

    xt_d = nc.dram_tensor("xt", [128, S], F32, kind="ExternalInput").ap()
    wq_d = nc.dram_tensor("wq", [128, HPC * 128], F32, kind="ExternalInput").ap()
    wk_d = nc.dram_tensor("wk", [128, HPC * 128], F32, kind="ExternalInput").ap()
    wv_d = nc.dram_tensor("wv", [128, HPC * 128], F32, kind="ExternalInput").ap()
    wo_d = nc.dram_tensor("wo", [128, HPC * 128], F32, kind="ExternalInput").ap()
    bq_d = nc.dram_tensor("bq", [128, HPC], F32, kind="ExternalInput").ap()
    bk_d = nc.dram_tensor("bk", [128, HPC], F32, kind="ExternalInput").ap()
    bvb_d = nc.dram_tensor("bvb", [128, HPC * 128], F32, kind="ExternalInput").ap()
    mk_d = nc.dram_tensor("masks", [128, 4 * 512], F32, kind="ExternalInput").ap()
    out_d = nc.dram_tensor("out_t", [128, S], F32, kind="ExternalOutput").ap()

    with ExitStack() as ctx:
        ctx.enter_context(
            nc.allow_low_precision(reason="fp32r matmul operands carry full fp32 bits")
        )
        tc = ctx.enter_context(tile.TileContext(nc))
        sb = ctx.enter_context(tc.tile_pool(name="sb", bufs=1))
        work = ctx.enter_context(tc.tile_pool(name="work", bufs=2))
        ptp = ctx.enter_context(tc.tile_pool(name="ptp", bufs=10))
        csp = ctx.enter_context(tc.tile_pool(name="csp", bufs=2))
        rcp = ctx.enter_context(tc.tile_pool(name="rcp", bufs=2))
        ps = ctx.enter_context(tc.tile_pool(name="ps", bufs=3, space="PSUM"))
        psc = ctx.enter_context(tc.tile_pool(name="psc", bufs=2, space="PSUM"))
        psr = ctx.enter_context(tc.tile_pool(name="psr", bufs=2, space="PSUM"))
        pso = ctx.enter_context(tc.tile_pool(name="pso", bufs=1, space="PSUM"))

        def load(name, dram_ap, shape, to_mm):
            """DMA a DRAM input to SBUF; in f32r mode bounce through DVE
            so the matmul operand is produced rounded-to-f32r."""
            t32 = sb.tile(shape, F32, tag=name + "32", name=name + "32")
            nc.sync.dma_start(t32[:], dram_ap[:])
            if not to_mm or MM is F32:
                return t32
            tr = sb.tile(shape, MM, tag=name, name=name)
            nc.vector.tensor_copy(tr[:], t32[:])
            return tr

        xt = load("xt", xt_d, [128, S], True)
        wq = load("wq", wq_d, [128, HPC * 128], True)
        wk = load("wk", wk_d, [128, HPC * 128], True)
        wv = load("wv", wv_d, [128, HPC * 128], True)
        wo = load("wo", wo_d, [128, HPC * 128], True)
        bq = load("bq", bq_d, [128, HPC], False)
        bk = load("bk", bk_d, [128, HPC], False)
        bvb = load("bvb", bvb_d, [128, HPC * 128], False)
        masks = load("masks", mk_d, [128, 4 * 512], True)

        ones_col32 = sb.tile([128, 1], F32, tag="ones_col32")
        nc.vector.memset(ones_col32[:], 1.0)
        ones_row32 = sb.tile([1, 128], F32, tag="ones_row32")
        nc.vector.memset(ones_row32[:], 1.0)
        if MM is F32:
            ones_col, ones_row = ones_col32, ones_row32
        else:
            ones_col = sb.tile([128, 1], MM, tag="ones_col")
            nc.vector.tensor_copy(ones_col[:], ones_col32[:])
            ones_row = sb.tile([1, 128], MM, tag="ones_row")
            nc.vector.tensor_copy(ones_row[:], ones_row32[:])

        out_acc = sb.tile([128, S], F32, tag="out_acc")

        for hp in range(HPC // 2):
            h0, h1 = 2 * hp, 2 * hp + 1
            kT = [None, None]
            qT = [None, None]
            # K^T / Q^T projections: [128 d, 2048 s] per head, bias fused
            # into the PSUM->SBUF copy (per-partition scalar add)
            for j, h in ((0, h0), (1, h1)):
                kT[j] = work.tile([128, S], MM, tag=f"kT{j}", name=f"kT{j}")
                qT[j] = work.tile([128, S], MM, tag=f"qT{j}", name=f"qT{j}")
                for sbk in range(4):
                    sl = slice(sbk * 512, (sbk + 1) * 512)
                    psK = ps.tile([128, 512], F32, tag="ps", name="psK")
                    nc.tensor.matmul(
                        psK[:], wk[:, h * 128 : (h + 1) * 128], xt[:, sl],
                        start=True, stop=True,
                    )
                    nc.vector.tensor_scalar_add(kT[j][:, sl], psK[:], bk[:, h : h + 1])
                    psQ = ps.tile([128, 512], F32, tag="ps", name="psQ")
                    nc.tensor.matmul(
                        psQ[:], wq[:, h * 128 : (h + 1) * 128], xt[:, sl],
                        start=True, stop=True,
                    )
                    nc.vector.tensor_scalar_add(qT[j][:, sl], psQ[:], bq[:, h : h + 1])
            # V for the head pair: [128 s, 256] tiles (two heads wide so the
            # moving dim is 256 and fp32r runs at 1 cycle/row)
            vsb = work.tile([128, 16 * 256], MM, tag="vsb", name="vsb")
            for st in range(16):
                psV = ps.tile([128, 256], F32, tag="ps", name="psV")
                nc.tensor.matmul(
                    psV[:], xt[:, st * 128 : (st + 1) * 128],
                    wv[:, hp * 256 : (hp + 1) * 256],
                    start=True, stop=True,
                )
                nc.vector.tensor_add(
                    vsb[:, st * 256 : (st + 1) * 256], psV[:],
                    bvb[:, hp * 256 : (hp + 1) * 256],
                )
            # attention: the two heads of the pair interleave per k-tile so
            # the PE always has the other head's independent matmuls queued
            # while one head's exp/mask chain drains on ACT/DVE
            for qb in range(4):
                qsl = slice(qb * 512, (qb + 1) * 512)
                nkt = 4 * (qb + 1)
                ctx_ps = [
                    psc.tile([128, 512], F32, tag="ctx", name="ctx_ps") for _ in range(2)
                ]
                row_ps = [
                    psr.tile([1, 512], F32, tag="row", name="row_ps") for _ in range(2)
                ]
                for kt in range(nkt):
                    for j in range(2):
                        s_ps = ps.tile([128, 512], F32, tag="ps", name="s_ps")
                        nc.tensor.matmul(
                            s_ps[:], kT[j][:, kt * 128 : (kt + 1) * 128], qT[j][:, qsl],
                            start=True, stop=True,
                        )
                        pT = ptp.tile([128, 512], MM, tag="pT", name="pT")
                        nc.scalar.activation(pT[:], s_ps[:], EXP, scale=float(SCALE))
                        di = kt - (nkt - 4)
                        if di >= 0:
                            nc.vector.tensor_mul(
                                pT[:], pT[:], masks[:, di * 512 : (di + 1) * 512]
                            )
                        nc.tensor.matmul(
                            ctx_ps[j][:],
                            vsb[:, kt * 256 + j * 128 : kt * 256 + j * 128 + 128],
                            pT[:], start=(kt == 0), stop=(kt == nkt - 1),
                        )
                        nc.tensor.matmul(
                            row_ps[j][:], ones_col[:], pT[:],
                            start=(kt == 0), stop=(kt == nkt - 1),
                        )
                for j, h in ((0, h0), (1, h1)):
                    recip = rcp.tile([1, 512], MM, tag="recip", name="recip")
                    nc.vector.reciprocal(recip[:], row_ps[j][:])
                    bc_ps = ps.tile([128, 512], F32, tag="ps", name="bc_ps")
                    nc.tensor.matmul(bc_ps[:], ones_row[:], recip[:], start=True, stop=True)
                    bc_s = csp.tile([128, 512], F32, tag="bcs", name="bc_s")
                    nc.scalar.copy(bc_s[:], bc_ps[:])
                    ctx_s = csp.tile([128, 512], MM, tag="cs", name="ctx_s")
                    nc.vector.tensor_mul(ctx_s[:], ctx_ps[j][:], bc_s[:])
                    o_ps = pso.tile([128, 512], F32, tag="o", name="o_ps")
                    nc.tensor.matmul(
                        o_ps[:], wo[:, h * 128 : (h + 1) * 128], ctx_s[:],
                        start=True, stop=True,
                    )
                    if hp == 0 and j == 0:
                        nc.vector.tensor_copy(out_acc[:, qsl], o_ps[:])
                    else:
                        nc.vector.tensor_add(out_acc[:, qsl], out_acc[:, qsl], o_ps[:])

        nc.sync.dma_start(out_d[:], out_acc[:])

    nc.compile()
    return nc


def _get_nc():
    if "nc" not in _CACHE:
        _CACHE["nc"] = _build_nc()
    return _CACHE["nc"]


def shard_inputs(query, Wq, bq, Wk, bk, Wv, bv, Wo, bo=None):
    query = np.asarray(query, np.float32)
    Wq, bq = np.asarray(Wq, np.float32), np.asarray(bq, np.float32)
    Wk, bk = np.asarray(Wk, np.float32), np.asarray(bk, np.float32)
    Wv, bv = np.asarray(Wv, np.float32), np.asarray(bv, np.float32)
    Wo = np.asarray(Wo, np.float32)

    # causal diag masks: masks[k, i*512 + q] = 1.0 iff i*128+k <= q
    kk = np.arange(128)[:, None]
    qq = np.arange(512)[None, :]
    masks = np.concatenate(
        [(kk + i * 128 <= qq).astype(np.float32) for i in range(4)], axis=1
    )

    in_maps = []
    for c in range(N_CORES):
        b, g = c // 2, c % 2
        hs = slice(g * HPC * 128, (g + 1) * HPC * 128)
        wo_l = (
            Wo[hs, :].reshape(HPC, 128, 128).transpose(1, 0, 2).reshape(128, HPC * 128)
        )
        in_maps.append(
            {
                "xt": np.ascontiguousarray(query[b].T),
                "wq": np.ascontiguousarray(Wq[:, hs]),
                "wk": np.ascontiguousarray(Wk[:, hs]),
                "wv": np.ascontiguousarray(Wv[:, hs]),
                "wo": np.ascontiguousarray(wo_l),
                "bq": np.ascontiguousarray(bq[hs].reshape(HPC, 128).T),
                "bk": np.ascontiguousarray(bk[hs].reshape(HPC, 128).T),
                "bvb": np.tile(bv[hs], (128, 1)),
                "masks": masks,
            }
        )
    return in_maps


def kernel(**inputs):
    _import_concourse()
    from concourse import bass_utils

    bo = np.asarray(inputs["bo"], np.float32)
    nc = _get_nc()
    in_maps = shard_inputs(**inputs)
    res = bass_utils.run_bass_kernel_spmd(nc, in_maps, list(range(N_CORES))).results
    out = np.empty((B, S, 128), np.float32)
    for b in range(B):
        out[b] = (res[2 * b]["out_t"] + res[2 * b + 1]["out_t"]).T + bo
    return out



# revision 11
# speedup vs baseline: 1.7014x; 1.7014x over previous
"""Multi-head causal self-attention (B=4, S=2048, H=16, D=128) on 8 TRN2 cores.

Sharding: core c = (batch b = c//2, head-group g = c%2 of 8 heads); host
sums the two head-group partials per batch and adds the bias (unshard).

Device math is restructured so Q/K/V projections and all four biases
disappear:
  scores[k,q] = k_k . q_q = x_k^T (Wk Wq^T) x_q + (Wk bq)^T x_k  [+ terms
  that are constant per q-column and cancel in softmax]. So one projected
  tensor per head, yT = wm^T x + gq with wm = Wq Wk^T, gq = Wk bq
  (host-precomputed), and scores tiles use raw x as the stationary.
  ctx = X P^T (X again stationary), out = sum_h N_h^T ctx_h with
  N_h = Wv_h Wo_h (host-precomputed); bv/bo fold into a host-side bias.

All SBUF operands are bf16 (PE streams 1 row/cycle, FWL weight loads, 2x
DVE); PSUM accumulation fp32. The softmax denominator comes from an
all-ones [128,128] stationary matmul accumulated in PSUM - it lands
pre-broadcast across partitions, so normalization is one full-width
reciprocal_approx_fast + one tensor_mul (no [1,512] ops, no bcast matmul).
Causal masking: per 512-q block, the last four k-tiles get a shared
[128,128] triangular band mask (the band is self-similar across diag
tiles); the two fully-masked column ranges of the final k-tile pair are
skipped in scores/exp/ctx/den entirely. exp runs as one fused [128,1024]
ACT op per k-tile pair. The out-projection accumulates all 8 heads into
one PSUM bank per q-block, DMA'd straight from PSUM."""

import os
import sys

import numpy as np

D = 128
B = 4
S = 2048
HPC = 8  # heads per core
N_CORES = 8
SCALE = 1.0 / np.sqrt(128.0)

_CACHE = {}


def _import_concourse():
    if "/opt/trn_rl_repo" not in sys.path and os.path.isdir("/opt/trn_rl_repo"):
        sys.path.insert(0, "/opt/trn_rl_repo")


def _build_nc():
    _import_concourse()
    from contextlib import ExitStack

    import concourse.mybir as mybir
    import concourse.tile as tile
    from concourse import bacc

    F32 = mybir.dt.float32
    BF = mybir.dt.bfloat16
    EXP = mybir.ActivationFunctionType.Exp
    IDENT = mybir.ActivationFunctionType.Identity

    nc = bacc.Bacc(trn_type="TRN2", target_bir_lowering=False, debug=False)

    xt_d = nc.dram_tensor("xt", [128, S], BF, kind="ExternalInput").ap()
    # xq: k-tile-major transpose of xt — block kt is X^T[kt*128:(kt+1)*128, :]
    # ([k, c] layout), the stationary orientation the ctx matmuls contract over
    xq_d = nc.dram_tensor("xq", [128, S], BF, kind="ExternalInput").ap()
    wm_d = nc.dram_tensor("wm", [128, HPC * 128], BF, kind="ExternalInput").ap()
    wn_d = nc.dram_tensor("wn", [128, HPC * 128], BF, kind="ExternalInput").ap()
    gq_d = nc.dram_tensor("gq", [128, HPC], F32, kind="ExternalInput").ap()
    band_d = nc.dram_tensor("band", [128, 128], BF, kind="ExternalInput").ap()
    zband_d = nc.dram_tensor("zband", [128, 256], BF, kind="ExternalInput").ap()
    out_d = nc.dram_tensor("out_t", [128, S], F32, kind="ExternalOutput").ap()

    with ExitStack() as ctx:
        ctx.enter_context(
            nc.allow_low_precision(reason="bf16 operands carry ample precision here")
        )
        tc = ctx.enter_context(tile.TileContext(nc))
        sb = ctx.enter_context(tc.tile_pool(name="sb", bufs=1))
        ptp = ctx.enter_context(tc.tile_pool(name="ptp", bufs=4))
        rcp = ctx.enter_context(tc.tile_pool(name="rcp", bufs=2))
        csp = ctx.enter_context(tc.tile_pool(name="csp", bufs=2))
        ps = ctx.enter_context(tc.tile_pool(name="ps", bufs=2, space="PSUM"))
        pctx = ctx.enter_context(tc.tile_pool(name="pctx", bufs=1, space="PSUM"))
        pden = ctx.enter_context(tc.tile_pool(name="pden", bufs=1, space="PSUM"))
        po = ctx.enter_context(tc.tile_pool(name="po", bufs=2, space="PSUM"))

        def load(name, dram_ap, shape, dt):
            t = sb.tile(shape, dt, tag=name, name=name)
            nc.sync.dma_start(t[:], dram_ap[:])
            return t

        xt = load("xt", xt_d, [128, S], BF)
        xq = load("xq", xq_d, [128, S], BF)
        wm = load("wm", wm_d, [128, HPC * 128], BF)
        wn = load("wn", wn_d, [128, HPC * 128], BF)
        gq = load("gq", gq_d, [128, HPC], F32)
        band = load("band", band_d, [128, 128], BF)
        zband = load("zband", zband_d, [128, 256], BF)

        ones32 = sb.tile([128, 128], F32, tag="ones32")
        nc.vector.memset(ones32[:], 1.0)
        ones = sb.tile([128, 128], BF, tag="ones")
        nc.vector.tensor_copy(ones[:], ones32[:])

        out_sb = sb.tile([128, S], F32, tag="out_sb")

        # yT for all heads: yt[:, h*S + q] = (Wk_h (Wq_h^T x_q + bq_h))[c']
        yt = sb.tile([128, HPC * S], BF, tag="yt")
        for h in range(HPC):
            for g2 in range(2):
                psY = ps.tile([128, 1024], F32, tag="ps", name="psY")
                for sbk in range(2):
                    sl = slice(g2 * 1024 + sbk * 512, g2 * 1024 + (sbk + 1) * 512)
                    nc.tensor.matmul(
                        psY[:, sbk * 512 : (sbk + 1) * 512],
                        wm[:, h * 128 : (h + 1) * 128],
                        xt[:, sl],
                        start=True, stop=True,
                    )
                ysl = slice(h * S + g2 * 1024, h * S + (g2 + 1) * 1024)
                if (2 * h + g2) % 2 == 0:
                    nc.scalar.activation(
                        yt[:, ysl], psY[:], IDENT, bias=gq[:, h : h + 1]
                    )
                else:
                    nc.vector.tensor_scalar_add(yt[:, ysl], psY[:], gq[:, h : h + 1])

        for qb in range(4):
            q0 = qb * 512
            npair = 2 * (qb + 1)
            o_ps = po.tile([128, 512], F32, tag="o", name="o_ps")
            pending = None  # (h, ctx_s) whose out-proj matmul is deferred so
            # the PE queue isn't blocked behind the DVE normalize chain
            for h in range(HPC):
                ctx_ps = pctx.tile([128, 512], F32, tag="ctx", name="ctx_ps")
                den_ps = pden.tile([128, 512], F32, tag="den", name="den_ps")
                for t in range(npair):
                    pair_a = t == npair - 2
                    pair_b = t == npair - 1
                    if t == 1 and pending is not None:
                        ph, pcs = pending
                        nc.tensor.matmul(
                            o_ps[:], wn[:, ph * 128 : (ph + 1) * 128], pcs[:],
                            start=(ph == 0), stop=False,
                        )
                        pending = None
                    x0 = xt[:, (2 * t) * 128 : (2 * t + 1) * 128]
                    x1 = xt[:, (2 * t + 1) * 128 : (2 * t + 2) * 128]
                    xq0 = xq[:, (2 * t) * 128 : (2 * t + 1) * 128]
                    xq1 = xq[:, (2 * t + 1) * 128 : (2 * t + 2) * 128]
                    s_ps = ps.tile([128, 1024], F32, tag="ps", name="s_ps")
                    pT = ptp.tile([128, 1024], BF, tag="pT", name="pT")
                    if not pair_b:
                        yq = yt[:, h * S + q0 : h * S + q0 + 512]
                        nc.tensor.matmul(s_ps[:, 0:512], x0, yq, start=True, stop=True)
                        nc.tensor.matmul(s_ps[:, 512:1024], x1, yq, start=True, stop=True)
                        nc.scalar.activation(pT[:], s_ps[:], EXP, scale=float(SCALE))
                        if pair_a:
                            nc.vector.tensor_mul(pT[:, 0:128], pT[:, 0:128], band[:])
                            nc.vector.tensor_mul(pT[:, 512:768], pT[:, 512:768], zband[:])
                        nc.tensor.matmul(
                            ctx_ps[:], xq0, pT[:, 0:512], start=(t == 0), stop=False
                        )
                        nc.tensor.matmul(
                            ctx_ps[:], xq1, pT[:, 512:1024], start=False, stop=False
                        )
                        nc.tensor.matmul(
                            den_ps[:], ones[:], pT[:, 0:512], start=(t == 0), stop=False
                        )
                        nc.tensor.matmul(
                            den_ps[:], ones[:], pT[:, 512:1024], start=False, stop=False
                        )
                    else:
                        # last k-tile pair: columns left of the diagonal band
                        # are fully masked - skip them in scores/exp/ctx/den
                        yq2 = yt[:, h * S + q0 + 256 : h * S + q0 + 512]
                        yq3 = yt[:, h * S + q0 + 384 : h * S + q0 + 512]
                        nc.tensor.matmul(s_ps[:, 256:512], x0, yq2, start=True, stop=True)
                        nc.tensor.matmul(s_ps[:, 896:1024], x1, yq3, start=True, stop=True)
                        nc.scalar.activation(
                            pT[:, 256:512], s_ps[:, 256:512], EXP, scale=float(SCALE)
                        )
                        nc.scalar.activation(
                            pT[:, 896:1024], s_ps[:, 896:1024], EXP, scale=float(SCALE)
                        )
                        nc.vector.tensor_mul(pT[:, 256:384], pT[:, 256:384], band[:])
                        nc.vector.tensor_mul(pT[:, 896:1024], pT[:, 896:1024], band[:])
                        nc.tensor.matmul(
                            ctx_ps[:, 256:512], xq0, pT[:, 256:512],
                            start=False, stop=False,
                        )
                        nc.tensor.matmul(
                            ctx_ps[:, 384:512], xq1, pT[:, 896:1024],
                            start=False, stop=True,
                        )
                        nc.tensor.matmul(
                            den_ps[:, 256:512], ones[:], pT[:, 256:512],
                            start=False, stop=False,
                        )
                        nc.tensor.matmul(
                            den_ps[:, 384:512], ones[:], pT[:, 896:1024],
                            start=False, stop=True,
                        )
                recip = rcp.tile([128, 512], F32, tag="recip", name="recip")
                nc.vector.reciprocal_approx_fast(recip[:], den_ps[:])
                ctx_s = csp.tile([128, 512], BF, tag="cs", name="ctx_s")
                nc.vector.tensor_mul(ctx_s[:], ctx_ps[:], recip[:])
                pending = (h, ctx_s)
            ph, pcs = pending
            nc.tensor.matmul(
                o_ps[:], wn[:, ph * 128 : (ph + 1) * 128], pcs[:],
                start=False, stop=True,
            )
            nc.vector.tensor_copy(out_sb[:, q0 : q0 + 512], o_ps[:])
            nc.sync.dma_start(out_d[:, q0 : q0 + 512], out_sb[:, q0 : q0 + 512])

    nc.compile()
    return nc


def _get_nc():
    if "nc" not in _CACHE:
        _CACHE["nc"] = _build_nc()
    return _CACHE["nc"]


def shard_inputs(query, Wq, bq, Wk, bk, Wv, bv, Wo, bo=None):
    import ml_dtypes

    BF = ml_dtypes.bfloat16
    query = np.asarray(query, np.float32)
    Wq, bq = np.asarray(Wq, np.float32), np.asarray(bq, np.float32)
    Wk = np.asarray(Wk, np.float32)
    Wv = np.asarray(Wv, np.float32)
    Wo = np.asarray(Wo, np.float32)

    band = (np.arange(128)[:, None] <= np.arange(128)[None, :]).astype(np.float32)
    zband = np.concatenate([np.zeros((128, 128), np.float32), band], axis=1)

    per_g = []
    for g in range(2):
        wm = np.empty((128, HPC * 128), np.float32)
        wn = np.empty((128, HPC * 128), np.float32)
        gq = np.empty((128, HPC), np.float32)
        for j in range(HPC):
            h = g * HPC + j
            hs = slice(h * 128, (h + 1) * 128)
            wm[:, j * 128 : (j + 1) * 128] = Wq[:, hs] @ Wk[:, hs].T
            wn[:, j * 128 : (j + 1) * 128] = Wv[:, hs] @ Wo[hs, :]
            gq[:, j] = Wk[:, hs] @ bq[hs]
        per_g.append(
            {
                "wm": np.ascontiguousarray(wm.astype(BF)),
                "wn": np.ascontiguousarray(wn.astype(BF)),
                "gq": np.ascontiguousarray(gq),
            }
        )

    in_maps = []
    for c in range(N_CORES):
        b, g = c // 2, c % 2
        in_maps.append(
            {
                "xt": np.ascontiguousarray(query[b].T.astype(BF)),
                "xq": np.ascontiguousarray(
                    query[b].reshape(16, 128, 128).transpose(1, 0, 2).reshape(128, S)
                    .astype(BF)
                ),
                "band": band.astype(BF),
                "zband": zband.astype(BF),
                **per_g[g],
            }
        )
    return in_maps


def kernel(**inputs):
    _import_concourse()
    from concourse import bass_utils

    bo = np.asarray(inputs["bo"], np.float32)
    bv = np.asarray(inputs["bv"], np.float32)
    Wo = np.asarray(inputs["Wo"], np.float32)
    bias_full = bo + Wo.T @ bv
    nc = _get_nc()
    in_maps = shard_inputs(**inputs)
    res = bass_utils.run_bass_kernel_spmd(nc, in_maps, list(range(N_CORES))).results
    out = np.empty((B, S, 128), np.float32)
    for b in range(B):
        out[b] = (res[2 * b]["out_t"] + res[2 * b + 1]["out_t"]).T + bias_full
    return out


# revision 12
# speedup vs baseline: 1.9937x; 1.1718x over previous
"""Multi-head causal self-attention (B=4, S=2048, H=16, D=128) on 8 TRN2 cores.

Sharding: core c = (batch b = c//2, head-group g = c%2 of 8 heads); host
sums the two head-group partials per batch and adds the bias (unshard).

Device math is restructured so Q/K/V projections and all four biases
disappear:
  scores[k,q] = k_k . q_q = x_k^T (Wk Wq^T) x_q + (Wk bq)^T x_k  [+ terms
  that are constant per q-column and cancel in softmax]. So one projected
  tensor per head, yT = wm^T x + gq with wm = Wq Wk^T, gq = Wk bq
  (host-precomputed), and scores tiles use raw x as the stationary.
  ctx = X P^T (X again stationary), out = sum_h N_h^T ctx_h with
  N_h = Wv_h Wo_h (host-precomputed); bv/bo fold into a host-side bias.

All SBUF operands are bf16 (PE streams 1 row/cycle, FWL weight loads, 2x
DVE); PSUM accumulation fp32. The softmax denominator comes from an
all-ones [128,128] stationary matmul accumulated in PSUM - it lands
pre-broadcast across partitions, so normalization is one full-width
reciprocal_approx_fast + one tensor_mul (no [1,512] ops, no bcast matmul).
Causal masking: per 512-q block, the last four k-tiles get a shared
[128,128] triangular band mask (the band is self-similar across diag
tiles); the two fully-masked column ranges of the final k-tile pair are
skipped in scores/exp/ctx/den entirely. exp runs as one fused [128,1024]
ACT op per k-tile pair. The out-projection accumulates all 8 heads into
one PSUM bank per q-block, DMA'd straight from PSUM."""

import os
import sys

import numpy as np

D = 128
B = 4
S = 2048
HPC = 8  # heads per core
N_CORES = 8
SCALE = 1.0 / np.sqrt(128.0)

_CACHE = {}


def _import_concourse():
    if "/opt/trn_rl_repo" not in sys.path and os.path.isdir("/opt/trn_rl_repo"):
        sys.path.insert(0, "/opt/trn_rl_repo")


def _build_nc():
    _import_concourse()
    from contextlib import ExitStack

    import concourse.mybir as mybir
    import concourse.tile as tile
    from concourse import bacc

    F32 = mybir.dt.float32
    BF = mybir.dt.bfloat16
    EXP = mybir.ActivationFunctionType.Exp
    IDENT = mybir.ActivationFunctionType.Identity

    nc = bacc.Bacc(trn_type="TRN2", target_bir_lowering=False, debug=False)

    xt_d = nc.dram_tensor("xt", [128, S], BF, kind="ExternalInput").ap()
    # xq: k-tile-major transpose of xt — block kt is X^T[kt*128:(kt+1)*128, :]
    # ([k, c] layout), the stationary orientation the ctx matmuls contract over
    xq_d = nc.dram_tensor("xq", [128, S], BF, kind="ExternalInput").ap()
    wm_d = nc.dram_tensor("wm", [128, HPC * 128], BF, kind="ExternalInput").ap()
    wn_d = nc.dram_tensor("wn", [128, HPC * 128], BF, kind="ExternalInput").ap()
    gq_d = nc.dram_tensor("gq", [128, HPC], F32, kind="ExternalInput").ap()
    band_d = nc.dram_tensor("band", [128, 128], BF, kind="ExternalInput").ap()
    zband_d = nc.dram_tensor("zband", [128, 256], BF, kind="ExternalInput").ap()
    out_d = nc.dram_tensor("out_t", [128, S], F32, kind="ExternalOutput").ap()

    with ExitStack() as ctx:
        ctx.enter_context(
            nc.allow_low_precision(reason="bf16 operands carry ample precision here")
        )
        tc = ctx.enter_context(tile.TileContext(nc))
        sb = ctx.enter_context(tc.tile_pool(name="sb", bufs=1))
        ptp = ctx.enter_context(tc.tile_pool(name="ptp", bufs=4))
        rcp = ctx.enter_context(tc.tile_pool(name="rcp", bufs=2))
        csp = ctx.enter_context(tc.tile_pool(name="csp", bufs=2))
        ps = ctx.enter_context(tc.tile_pool(name="ps", bufs=2, space="PSUM"))
        pctx = ctx.enter_context(tc.tile_pool(name="pctx", bufs=2, space="PSUM"))
        pden = ctx.enter_context(tc.tile_pool(name="pden", bufs=1, space="PSUM"))
        po = ctx.enter_context(tc.tile_pool(name="po", bufs=1, space="PSUM"))

        def load(name, dram_ap, shape, dt):
            t = sb.tile(shape, dt, tag=name, name=name)
            nc.sync.dma_start(t[:], dram_ap[:])
            return t

        xt = load("xt", xt_d, [128, S], BF)
        xq = load("xq", xq_d, [128, S], BF)
        wm = load("wm", wm_d, [128, HPC * 128], BF)
        wn = load("wn", wn_d, [128, HPC * 128], BF)
        gq = load("gq", gq_d, [128, HPC], F32)
        band = load("band", band_d, [128, 128], BF)
        zband = load("zband", zband_d, [128, 256], BF)

        ones32 = sb.tile([128, 128], F32, tag="ones32")
        nc.vector.memset(ones32[:], 1.0)
        ones = sb.tile([128, 128], BF, tag="ones")
        nc.vector.tensor_copy(ones[:], ones32[:])

        out_sb = sb.tile([128, S], F32, tag="out_sb")

        # yT for all heads: yt[:, h*S + q] = (Wk_h (Wq_h^T x_q + bq_h))[c']
        yt = sb.tile([128, HPC * S], BF, tag="yt")
        for h in range(HPC):
            for g2 in range(2):
                psY = ps.tile([128, 1024], F32, tag="ps", name="psY")
                for sbk in range(2):
                    sl = slice(g2 * 1024 + sbk * 512, g2 * 1024 + (sbk + 1) * 512)
                    nc.tensor.matmul(
                        psY[:, sbk * 512 : (sbk + 1) * 512],
                        wm[:, h * 128 : (h + 1) * 128],
                        xt[:, sl],
                        start=True, stop=True,
                    )
                ysl = slice(h * S + g2 * 1024, h * S + (g2 + 1) * 1024)
                if (2 * h + g2) % 2 == 0:
                    nc.scalar.activation(
                        yt[:, ysl], psY[:], IDENT, bias=gq[:, h : h + 1]
                    )
                else:
                    nc.vector.tensor_scalar_add(yt[:, ysl], psY[:], gq[:, h : h + 1])

        for qb in range(4):
            q0 = qb * 512
            npair = 2 * (qb + 1)
            o_ps = po.tile([128, 512], F32, tag="o", name="o_ps")
            ctxden = {}
            prev = None       # (h, t, pT) whose ctx/den matmuls are deferred
            pending_out = None  # (h, ctx_s) whose out-proj matmul is deferred
            # one-slot software pipeline: emit scores/exp for pair i+1 before
            # the ctx/den matmuls of pair i, so the PE never queues behind exp

            def emit_cd(h, t, pT):
                if t == 0:
                    ctxden[h] = (
                        pctx.tile([128, 512], F32, tag="ctx", name="ctx_ps"),
                        pden.tile([128, 512], F32, tag="den", name="den_ps"),
                    )
                ctx_ps, den_ps = ctxden[h]
                xq0 = xq[:, (2 * t) * 128 : (2 * t + 1) * 128]
                xq1 = xq[:, (2 * t + 1) * 128 : (2 * t + 2) * 128]
                if t < npair - 1:
                    nc.tensor.matmul(
                        ctx_ps[:], xq0, pT[:, 0:512], start=(t == 0), stop=False
                    )
                    nc.tensor.matmul(
                        ctx_ps[:], xq1, pT[:, 512:1024], start=False, stop=False
                    )
                    nc.tensor.matmul(
                        den_ps[:], ones[:], pT[:, 0:512], start=(t == 0), stop=False
                    )
                    nc.tensor.matmul(
                        den_ps[:], ones[:], pT[:, 512:1024], start=False, stop=False
                    )
                else:
                    nc.tensor.matmul(
                        ctx_ps[:, 256:512], xq0, pT[:, 256:512],
                        start=False, stop=False,
                    )
                    nc.tensor.matmul(
                        ctx_ps[:, 384:512], xq1, pT[:, 896:1024],
                        start=False, stop=True,
                    )
                    nc.tensor.matmul(
                        den_ps[:, 256:512], ones[:], pT[:, 256:512],
                        start=False, stop=False,
                    )
                    nc.tensor.matmul(
                        den_ps[:, 384:512], ones[:], pT[:, 896:1024],
                        start=False, stop=True,
                    )

            def emit_norm(h):
                ctx_ps, den_ps = ctxden.pop(h)
                recip = rcp.tile([128, 512], F32, tag="recip", name="recip")
                nc.vector.reciprocal_approx_fast(recip[:], den_ps[:])
                ctx_s = csp.tile([128, 512], BF, tag="cs", name="ctx_s")
                nc.vector.tensor_mul(ctx_s[:], ctx_ps[:], recip[:])
                return (h, ctx_s)

            def emit_out(h, ctx_s):
                nc.tensor.matmul(
                    o_ps[:], wn[:, h * 128 : (h + 1) * 128], ctx_s[:],
                    start=(h == 0), stop=(h == HPC - 1),
                )

            for h in range(HPC):
                for t in range(npair):
                    pair_a = t == npair - 2
                    pair_b = t == npair - 1
                    x0 = xt[:, (2 * t) * 128 : (2 * t + 1) * 128]
                    x1 = xt[:, (2 * t + 1) * 128 : (2 * t + 2) * 128]
                    s_ps = ps.tile([128, 1024], F32, tag="ps", name="s_ps")
                    pT = ptp.tile([128, 1024], BF, tag="pT", name="pT")
                    if not pair_b:
                        yq = yt[:, h * S + q0 : h * S + q0 + 512]
                        nc.tensor.matmul(s_ps[:, 0:512], x0, yq, start=True, stop=True)
                        nc.tensor.matmul(s_ps[:, 512:1024], x1, yq, start=True, stop=True)
                        nc.scalar.activation(pT[:], s_ps[:], EXP, scale=float(SCALE))
                        if pair_a:
                            nc.vector.tensor_mul(pT[:, 0:128], pT[:, 0:128], band[:])
                            nc.vector.tensor_mul(pT[:, 512:768], pT[:, 512:768], zband[:])
                    else:
                        yq2 = yt[:, h * S + q0 + 256 : h * S + q0 + 512]
                        yq3 = yt[:, h * S + q0 + 384 : h * S + q0 + 512]
                        nc.tensor.matmul(s_ps[:, 256:512], x0, yq2, start=True, stop=True)
                        nc.tensor.matmul(s_ps[:, 896:1024], x1, yq3, start=True, stop=True)
                        nc.scalar.activation(
                            pT[:, 256:512], s_ps[:, 256:512], EXP, scale=float(SCALE)
                        )
                        nc.scalar.activation(
                            pT[:, 896:1024], s_ps[:, 896:1024], EXP, scale=float(SCALE)
                        )
                        nc.vector.tensor_mul(pT[:, 256:384], pT[:, 256:384], band[:])
                        nc.vector.tensor_mul(pT[:, 896:1024], pT[:, 896:1024], band[:])
                    if pending_out is not None:
                        emit_out(*pending_out)
                        pending_out = None
                    if prev is not None:
                        ph, pt_, ppT = prev
                        emit_cd(ph, pt_, ppT)
                        if pt_ == npair - 1:
                            pending_out = emit_norm(ph)
                    prev = (h, t, pT)
            ph, pt_, ppT = prev
            emit_cd(ph, pt_, ppT)
            emit_out(*emit_norm(ph))
            nc.vector.tensor_copy(out_sb[:, q0 : q0 + 512], o_ps[:])
            nc.sync.dma_start(out_d[:, q0 : q0 + 512], out_sb[:, q0 : q0 + 512])

    nc.compile()
    return nc


def _get_nc():
    if "nc" not in _CACHE:
        _CACHE["nc"] = _build_nc()
    return _CACHE["nc"]


def shard_inputs(query, Wq, bq, Wk, bk, Wv, bv, Wo, bo=None):
    import ml_dtypes

    BF = ml_dtypes.bfloat16
    query = np.asarray(query, np.float32)
    Wq, bq = np.asarray(Wq, np.float32), np.asarray(bq, np.float32)
    Wk = np.asarray(Wk, np.float32)
    Wv = np.asarray(Wv, np.float32)
    Wo = np.asarray(Wo, np.float32)

    band = (np.arange(128)[:, None] <= np.arange(128)[None, :]).astype(np.float32)
    zband = np.concatenate([np.zeros((128, 128), np.float32), band], axis=1)

    per_g = []
    for g in range(2):
        wm = np.empty((128, HPC * 128), np.float32)
        wn = np.empty((128, HPC * 128), np.float32)
        gq = np.empty((128, HPC), np.float32)
        for j in range(HPC):
            h = g * HPC + j
            hs = slice(h * 128, (h + 1) * 128)
            wm[:, j * 128 : (j + 1) * 128] = Wq[:, hs] @ Wk[:, hs].T
            wn[:, j * 128 : (j + 1) * 128] = Wv[:, hs] @ Wo[hs, :]
            gq[:, j] = Wk[:, hs] @ bq[hs]
        per_g.append(
            {
                "wm": np.ascontiguousarray(wm.astype(BF)),
                "wn": np.ascontiguousarray(wn.astype(BF)),
                "gq": np.ascontiguousarray(gq),
            }
        )

    in_maps = []
    for c in range(N_CORES):
        b, g = c // 2, c % 2
        in_maps.append(
            {
                "xt": np.ascontiguousarray(query[b].T.astype(BF)),
                "xq": np.ascontiguousarray(
                    query[b].reshape(16, 128, 128).transpose(1, 0, 2).reshape(128, S)
                    .astype(BF)
                ),
                "band": band.astype(BF),
                "zband": zband.astype(BF),
                **per_g[g],
            }
        )
    return in_maps


def kernel(**inputs):
    _import_concourse()
    from concourse import bass_utils

    bo = np.asarray(inputs["bo"], np.float32)
    bv = np.asarray(inputs["bv"], np.float32)
    Wo = np.asarray(inputs["Wo"], np.float32)
    bias_full = bo + Wo.T @ bv
    nc = _get_nc()
    in_maps = shard_inputs(**inputs)
    res = bass_utils.run_bass_kernel_spmd(nc, in_maps, list(range(N_CORES))).results
    out = np.empty((B, S, 128), np.float32)
    for b in range(B):
        out[b] = (res[2 * b]["out_t"] + res[2 * b + 1]["out_t"]).T + bias_full
    return out


# revision 15
# speedup vs baseline: 2.0424x; 1.0245x over previous
"""Multi-head causal self-attention (B=4, S=2048, H=16, D=128) on 8 TRN2 cores.

Sharding: core c = (batch b = c//2, head-group g = c%2 of 8 heads); host
sums the two head-group partials per batch and adds the bias (unshard).

Device math is restructured so Q/K/V projections and all four biases
disappear:
  scores[k,q] = k_k . q_q = x_k^T (Wk Wq^T) x_q + (Wk bq)^T x_k  [+ terms
  that are constant per q-column and cancel in softmax]. So one projected
  tensor per head, yT = wm^T x + gq with wm = Wq Wk^T, gq = Wk bq
  (host-precomputed), and scores tiles use raw x as the stationary.
  ctx = X P^T (X again stationary), out = sum_h N_h^T ctx_h with
  N_h = Wv_h Wo_h (host-precomputed); bv/bo fold into a host-side bias.

All SBUF operands are bf16 (PE streams 1 row/cycle, FWL weight loads, 2x
DVE); PSUM accumulation fp32. The softmax denominator comes from an
all-ones [128,128] stationary matmul accumulated in PSUM - it lands
pre-broadcast across partitions, so normalization is one full-width
reciprocal_approx_fast + one tensor_mul (no [1,512] ops, no bcast matmul).
Causal masking: per 512-q block, the last four k-tiles get a shared
[128,128] triangular band mask (the band is self-similar across diag
tiles); the two fully-masked column ranges of the final k-tile pair are
skipped in scores/exp/ctx/den entirely. exp runs as one fused [128,1024]
ACT op per k-tile pair. The out-projection accumulates all 8 heads into
one PSUM bank per q-block, DMA'd straight from PSUM."""

import os
import sys

import numpy as np

D = 128
B = 4
S = 2048
HPC = 8  # heads per core
N_CORES = 8
SCALE = 1.0 / np.sqrt(128.0)

_CACHE = {}


def _import_concourse():
    if "/opt/trn_rl_repo" not in sys.path and os.path.isdir("/opt/trn_rl_repo"):
        sys.path.insert(0, "/opt/trn_rl_repo")


def _build_nc():
    _import_concourse()
    from contextlib import ExitStack

    import concourse.mybir as mybir
    import concourse.tile as tile
    from concourse import bacc

    F32 = mybir.dt.float32
    BF = mybir.dt.bfloat16
    EXP = mybir.ActivationFunctionType.Exp
    IDENT = mybir.ActivationFunctionType.Identity

    nc = bacc.Bacc(trn_type="TRN2", target_bir_lowering=False, debug=False)

    xt_d = nc.dram_tensor("xt", [128, S], BF, kind="ExternalInput").ap()
    # xq: k-tile-major transpose of xt — block kt is X^T[kt*128:(kt+1)*128, :]
    # ([k, c] layout), the stationary orientation the ctx matmuls contract over
    xq_d = nc.dram_tensor("xq", [128, S], BF, kind="ExternalInput").ap()
    wm_d = nc.dram_tensor("wm", [128, HPC * 128], BF, kind="ExternalInput").ap()
    wn_d = nc.dram_tensor("wn", [128, HPC * 128], BF, kind="ExternalInput").ap()
    gq_d = nc.dram_tensor("gq", [128, HPC], F32, kind="ExternalInput").ap()
    band_d = nc.dram_tensor("band", [128, 128], BF, kind="ExternalInput").ap()
    out_d = nc.dram_tensor("out_t", [128, S], F32, kind="ExternalOutput").ap()

    with ExitStack() as ctx:
        ctx.enter_context(
            nc.allow_low_precision(reason="bf16 operands carry ample precision here")
        )
        tc = ctx.enter_context(tile.TileContext(nc))
        sb = ctx.enter_context(tc.tile_pool(name="sb", bufs=1))
        ptp = ctx.enter_context(tc.tile_pool(name="ptp", bufs=4))
        rcp = ctx.enter_context(tc.tile_pool(name="rcp", bufs=2))
        csp = ctx.enter_context(tc.tile_pool(name="csp", bufs=2))
        ps = ctx.enter_context(tc.tile_pool(name="ps", bufs=2, space="PSUM"))
        pctx = ctx.enter_context(tc.tile_pool(name="pctx", bufs=2, space="PSUM"))
        pden = ctx.enter_context(tc.tile_pool(name="pden", bufs=1, space="PSUM"))
        po = ctx.enter_context(tc.tile_pool(name="po", bufs=1, space="PSUM"))

        def load(name, dram_ap, shape, dt):
            t = sb.tile(shape, dt, tag=name, name=name)
            nc.sync.dma_start(t[:], dram_ap[:])
            return t

        # wm/xt/gq first: the yT matmuls and drains need them immediately
        wm = load("wm", wm_d, [128, HPC * 128], BF)
        xt = load("xt", xt_d, [128, S], BF)
        gq = load("gq", gq_d, [128, HPC], F32)
        xq = load("xq", xq_d, [128, S], BF)
        wn = load("wn", wn_d, [128, HPC * 128], BF)
        band = load("band", band_d, [128, 128], BF)

        ones32 = sb.tile([128, 128], F32, tag="ones32")
        nc.vector.memset(ones32[:], 1.0)
        ones = sb.tile([128, 128], BF, tag="ones")
        nc.vector.tensor_copy(ones[:], ones32[:])

        out_sb = sb.tile([128, S], F32, tag="out_sb")

        # yT per head (separate tiles so attention on head h only waits on
        # head h's drains): yt[h][:, q] = (Wk_h (Wq_h^T x_q + bq_h))[c']
        yt = [sb.tile([128, S], BF, tag=f"yt{h}", name=f"yt{h}") for h in range(HPC)]
        for h in range(HPC):
            for g2 in range(2):
                psY = ps.tile([128, 1024], F32, tag="ps", name="psY")
                for sbk in range(2):
                    sl = slice(g2 * 1024 + sbk * 512, g2 * 1024 + (sbk + 1) * 512)
                    nc.tensor.matmul(
                        psY[:, sbk * 512 : (sbk + 1) * 512],
                        wm[:, h * 128 : (h + 1) * 128],
                        xt[:, sl],
                        start=True, stop=True,
                    )
                ysl = slice(g2 * 1024, (g2 + 1) * 1024)
                if (2 * h + g2) % 2 == 0:
                    nc.scalar.activation(
                        yt[h][:, ysl], psY[:], IDENT, bias=gq[:, h : h + 1]
                    )
                else:
                    nc.vector.tensor_scalar_add(yt[h][:, ysl], psY[:], gq[:, h : h + 1])

        for qb in range(4):
            q0 = qb * 512
            npair = 2 * (qb + 1)
            o_ps = po.tile([128, 512], F32, tag="o", name="o_ps")
            ctxden = {}
            prev = None       # (h, t, pT) whose ctx/den matmuls are deferred
            pending_out = None  # (h, ctx_s) whose out-proj matmul is deferred
            # one-slot software pipeline: emit scores/exp for pair i+1 before
            # the ctx/den matmuls of pair i, so the PE never queues behind exp

            def emit_cd(h, t, pT):
                if t == 0:
                    ctxden[h] = (
                        pctx.tile([128, 512], F32, tag="ctx", name="ctx_ps"),
                        pden.tile([128, 512], F32, tag="den", name="den_ps"),
                    )
                ctx_ps, den_ps = ctxden[h]
                xq0 = xq[:, (2 * t) * 128 : (2 * t + 1) * 128]
                xq1 = xq[:, (2 * t + 1) * 128 : (2 * t + 2) * 128]
                if t < npair - 1:
                    pa = t == npair - 2
                    lo = 640 if pa else 512  # di1: cols [512:640] fully masked
                    qlo = 128 if pa else 0
                    nc.tensor.matmul(
                        ctx_ps[:], xq0, pT[:, 0:512], start=(t == 0), stop=False
                    )
                    nc.tensor.matmul(
                        ctx_ps[:, qlo:512], xq1, pT[:, lo:1024], start=False, stop=False
                    )
                    nc.tensor.matmul(
                        den_ps[:], ones[:], pT[:, 0:512], start=(t == 0), stop=False
                    )
                    nc.tensor.matmul(
                        den_ps[:, qlo:512], ones[:], pT[:, lo:1024], start=False, stop=False
                    )
                else:
                    nc.tensor.matmul(
                        ctx_ps[:, 256:512], xq0, pT[:, 256:512],
                        start=False, stop=False,
                    )
                    nc.tensor.matmul(
                        ctx_ps[:, 384:512], xq1, pT[:, 896:1024],
                        start=False, stop=True,
                    )
                    nc.tensor.matmul(
                        den_ps[:, 256:512], ones[:], pT[:, 256:512],
                        start=False, stop=False,
                    )
                    nc.tensor.matmul(
                        den_ps[:, 384:512], ones[:], pT[:, 896:1024],
                        start=False, stop=True,
                    )

            def emit_norm(h):
                ctx_ps, den_ps = ctxden.pop(h)
                recip = rcp.tile([128, 512], F32, tag="recip", name="recip")
                nc.vector.reciprocal_approx_fast(recip[:], den_ps[:])
                ctx_s = csp.tile([128, 512], BF, tag="cs", name="ctx_s")
                nc.vector.tensor_mul(ctx_s[:], ctx_ps[:], recip[:])
                return (h, ctx_s)

            def emit_out(h, ctx_s):
                nc.tensor.matmul(
                    o_ps[:], wn[:, h * 128 : (h + 1) * 128], ctx_s[:],
                    start=(h == 0), stop=(h == HPC - 1),
                )

            for h in range(HPC):
                for t in range(npair):
                    pair_a = t == npair - 2
                    pair_b = t == npair - 1
                    x0 = xt[:, (2 * t) * 128 : (2 * t + 1) * 128]
                    x1 = xt[:, (2 * t + 1) * 128 : (2 * t + 2) * 128]
                    s_ps = ps.tile([128, 1024], F32, tag="ps", name="s_ps")
                    pT = ptp.tile([128, 1024], BF, tag="pT", name="pT")
                    if not pair_b:
                        yq = yt[h][:, q0 : q0 + 512]
                        nc.tensor.matmul(s_ps[:, 0:512], x0, yq, start=True, stop=True)
                        nc.tensor.matmul(s_ps[:, 512:1024], x1, yq, start=True, stop=True)
                        nc.scalar.activation(pT[:], s_ps[:], EXP, scale=float(SCALE))
                        if pair_a:
                            nc.vector.tensor_mul(pT[:, 0:128], pT[:, 0:128], band[:])
                            nc.vector.tensor_mul(pT[:, 640:768], pT[:, 640:768], band[:])
                    else:
                        yq2 = yt[h][:, q0 + 256 : q0 + 512]
                        yq3 = yt[h][:, q0 + 384 : q0 + 512]
                        nc.tensor.matmul(s_ps[:, 256:512], x0, yq2, start=True, stop=True)
                        nc.tensor.matmul(s_ps[:, 896:1024], x1, yq3, start=True, stop=True)
                        nc.scalar.activation(
                            pT[:, 256:512], s_ps[:, 256:512], EXP, scale=float(SCALE)
                        )
                        nc.scalar.activation(
                            pT[:, 896:1024], s_ps[:, 896:1024], EXP, scale=float(SCALE)
                        )
                        nc.vector.tensor_mul(pT[:, 256:384], pT[:, 256:384], band[:])
                        nc.vector.tensor_mul(pT[:, 896:1024], pT[:, 896:1024], band[:])
                    if pending_out is not None:
                        emit_out(*pending_out)
                        pending_out = None
                    if prev is not None:
                        ph, pt_, ppT = prev
                        emit_cd(ph, pt_, ppT)
                        if pt_ == npair - 1:
                            pending_out = emit_norm(ph)
                    prev = (h, t, pT)
            ph, pt_, ppT = prev
            emit_cd(ph, pt_, ppT)
            emit_out(*emit_norm(ph))
            nc.vector.tensor_copy(out_sb[:, q0 : q0 + 512], o_ps[:])
            nc.sync.dma_start(out_d[:, q0 : q0 + 512], out_sb[:, q0 : q0 + 512])

    nc.compile()
    return nc


def _get_nc():
    if "nc" not in _CACHE:
        _CACHE["nc"] = _build_nc()
    return _CACHE["nc"]


def shard_inputs(query, Wq, bq, Wk, bk, Wv, bv, Wo, bo=None):
    import ml_dtypes

    BF = ml_dtypes.bfloat16
    query = np.asarray(query, np.float32)
    Wq, bq = np.asarray(Wq, np.float32), np.asarray(bq, np.float32)
    Wk = np.asarray(Wk, np.float32)
    Wv = np.asarray(Wv, np.float32)
    Wo = np.asarray(Wo, np.float32)

    band = (np.arange(128)[:, None] <= np.arange(128)[None, :]).astype(np.float32)

    per_g = []
    for g in range(2):
        wm = np.empty((128, HPC * 128), np.float32)
        wn = np.empty((128, HPC * 128), np.float32)
        gq = np.empty((128, HPC), np.float32)
        for j in range(HPC):
            h = g * HPC + j
            hs = slice(h * 128, (h + 1) * 128)
            wm[:, j * 128 : (j + 1) * 128] = Wq[:, hs] @ Wk[:, hs].T
            wn[:, j * 128 : (j + 1) * 128] = Wv[:, hs] @ Wo[hs, :]
            gq[:, j] = Wk[:, hs] @ bq[hs]
        per_g.append(
            {
                "wm": np.ascontiguousarray(wm.astype(BF)),
                "wn": np.ascontiguousarray(wn.astype(BF)),
                "gq": np.ascontiguousarray(gq),
            }
        )

    in_maps = []
    for c in range(N_CORES):
        b, g = c // 2, c % 2
        in_maps.append(
            {
                "xt": np.ascontiguousarray(query[b].T.astype(BF)),
                "xq": np.ascontiguousarray(
                    query[b].reshape(16, 128, 128).transpose(1, 0, 2).reshape(128, S)
                    .astype(BF)
                ),
                "band": band.astype(BF),
                **per_g[g],
            }
        )
    return in_maps


def kernel(**inputs):
    _import_concourse()
    from concourse import bass_utils

    bo = np.asarray(inputs["bo"], np.float32)
    bv = np.asarray(inputs["bv"], np.float32)
    Wo = np.asarray(inputs["Wo"], np.float32)
    bias_full = bo + Wo.T @ bv
    nc = _get_nc()
    in_maps = shard_inputs(**inputs)
    res = bass_utils.run_bass_kernel_spmd(nc, in_maps, list(range(N_CORES))).results
    out = np.empty((B, S, 128), np.float32)
    for b in range(B):
        out[b] = (res[2 * b]["out_t"] + res[2 * b + 1]["out_t"]).T + bias_full
    return out
